# revision 28
# baseline (speedup 1.0000x reference)
"""DiffMoE MLP (8 experts, capacity 1.0) — expert-parallel across 8 TRN2 NeuronCores.

Contract: kernel(**full_inputs) -> full output (4, 2048, 1024) f32.

Strategy (expert-parallel, sharding_hint):
  host   : gating scores + per-expert top-k (bit-identical jnp ops to the
           reference), token gather + fp32 LayerNorm of the gathered tokens,
           weight re-layout into the PE stationary-block format plus hi/lo
           fp8 splitting, and the final topk-weight scale + scatter-add
           combine.
  device : core e owns expert e.  Both GEMMs run as 3-pass compensated fp8
           with DoubleRow (128x256 virtual PE array):
               y @ W ~= y_hi @ W_hi + y_hi @ W_lo + y_lo @ W_hi
           with hi = e4m3(x), lo = e5m2(x - hi), all three passes
           accumulating into the same fp32 PSUM group.  The dropped
           lo@lo term is second order (~2^-8); fc2's activation-residual
           pass covers 12/16 contraction pairs (measured rel err 0.012
           vs the 2e-2 gate) at ~0.71x the bf16 matmul cost.
           fc1 -> bf16 gelu (ScalarE) -> DVE hi/lo split -> fc2 -> bias.
           All tensors take the same SBUF/HBM bytes as the bf16 version.
"""

import sys

for _p in ("/opt/trn_rl_repo", "/root/.axon_site/_ro/trn_rl_repo"):
    if _p not in sys.path:
        sys.path.append(_p)

import numpy as np
import ml_dtypes

import concourse.bass as bass
import concourse.bacc as bacc
import concourse.tile as tile
from concourse import mybir
from concourse.bass_utils import run_bass_kernel_spmd

BF16 = ml_dtypes.bfloat16
E4 = ml_dtypes.float8_e4m3
E5 = ml_dtypes.float8_e5m2

D = 1024          # embed dim
F = 4096          # hidden dim
N_EXP = 8         # experts == cores
BS = 8192         # tokens
K_TOK = 1024      # tokens kept per expert (BS * capacity / n_exp)
LN_EPS = 1e-5

P = 128
KD = D // P       # 8   d-chunks
KF = F // P       # 32  f-chunks
KD2 = KD // 2     # 4   DoubleRow d-chunk pairs
KF2 = KF // 2     # 16  DoubleRow f-chunk pairs
TH = 512          # moving-operand token tile (one PSUM bank)
NT = K_TOK // TH  # 2   token halves

_NC_CACHE = {}


def _build_nc(debug=False, reps=1):
    nc = bacc.Bacc("TRN2", target_bir_lowering=False, debug=debug)
    f32 = mybir.dt.float32
    bf16 = mybir.dt.bfloat16
    e4 = mybir.dt.float8e4
    e5 = mybir.dt.float8e5
    DR = mybir.MatmulPerfMode.DoubleRow

    ynh = nc.dram_tensor("ynh", [KD, P, K_TOK], e4, kind="ExternalInput")
    ynl = nc.dram_tensor("ynl", [KD, P, K_TOK], e5, kind="ExternalInput")
    w1h = nc.dram_tensor("w1h", [KF, P, KD * P], e4, kind="ExternalInput")
    w1l = nc.dram_tensor("w1l", [KF, P, KD * P], e5, kind="ExternalInput")
    w2h = nc.dram_tensor("w2h", [KF, P, KD * P], e4, kind="ExternalInput")
    w2l = nc.dram_tensor("w2l", [KF, P, KD * P], e5, kind="ExternalInput")
    b1r = nc.dram_tensor("b1r", [P, KF], f32, kind="ExternalInput")
    b2r = nc.dram_tensor("b2r", [P, KD], f32, kind="ExternalInput")
    ot = nc.dram_tensor("ot", [D, K_TOK], f32, kind="ExternalOutput")

    with tile.TileContext(nc) as tc:
        with (
            tc.tile_pool(name="singles", bufs=1) as singles,
            tc.tile_pool(name="big", bufs=1) as big,
            tc.tile_pool(name="w1p", bufs=4) as w1p,
            tc.tile_pool(name="tmpp", bufs=4) as tmpp,
            tc.tile_pool(name="outp", bufs=4) as outp,
            tc.tile_pool(name="psum", bufs=8, space="PSUM") as psum,
        ):
          for _rep in range(reps):
            # ---- startup: DMA front-end costs ~0.6 us per descriptor and
            # transfers serialize, so issue FEW DMAs in strict need-order.
            # Pass A of the first m-pair group needs w1h[0:2] + ynh; the lo
            # operands (pass B/C) stream in behind. ----
            ynh_sb = big.tile([P, KD, K_TOK], e4)
            ynl_sb = big.tile([P, KD, K_TOK], e5)
            hh_sb = big.tile([P, KF, K_TOK], e4)
            hl_sb = big.tile([P, KF, K_TOK], e5)
            w2h_sb = big.tile([P, KF, KD * P], e4)
            w2l_sb = big.tile([P, KF, KD * P], e5)

            def w1_group_load(g, name):
                hi = w1p.tile([P, 2, KD, P], e4, tag="w1hg", name=f"{name}h")
                nc.sync.dma_start(
                    out=hi, in_=w1h[2 * g:2 * g + 2].rearrange("a b c -> b a c"))
                lo = w1p.tile([P, 2, KD, P], e5, tag="w1lg", name=f"{name}l")
                nc.sync.dma_start(
                    out=lo, in_=w1l[2 * g:2 * g + 2].rearrange("a b c -> b a c"))
                return hi, lo

            # warm-up: the PE clock ramps from its first busy moment
            # (LOW -> MID -> full after ~3 us) and never resets, so a few
            # matmuls on memset garbage while the first DMAs are in flight
            # buy full clock for every real matmul
            warm = singles.tile([P, P], bf16)
            nc.vector.memset(warm[:], 0)
            wps = psum.tile([P, P], f32, tag="ps", name="warmps")
            for _ in range(3):
                nc.tensor.matmul(wps, warm[:], warm[:],
                                 start=True, stop=True)

            w1hg0 = w1p.tile([P, 2, KD, P], e4, tag="w1hg", name="w1hg0")
            nc.sync.dma_start(
                out=w1hg0, in_=w1h[0:2].rearrange("a b c -> b a c"))
            for kp in range(KD2):
                nc.sync.dma_start(
                    out=ynh_sb[:, 2 * kp:2 * kp + 2, :],
                    in_=ynh[2 * kp:2 * kp + 2].rearrange("a b c -> b a c"))
            w1lg0 = w1p.tile([P, 2, KD, P], e5, tag="w1lg", name="w1lg0")
            nc.sync.dma_start(
                out=w1lg0, in_=w1l[0:2].rearrange("a b c -> b a c"))
            nc.sync.dma_start(out=ynl_sb[:, 0:KD2, :],
                              in_=ynl[0:KD2].rearrange("a b c -> b a c"))
            nc.sync.dma_start(out=ynl_sb[:, KD2:, :],
                              in_=ynl[KD2:].rearrange("a b c -> b a c"))
            b1_sb = singles.tile([P, KF], f32)
            nc.sync.dma_start(out=b1_sb, in_=b1r[:])
            b2_sb = singles.tile([P, KD], f32)
            nc.sync.dma_start(out=b2_sb, in_=b2r[:])

            # ---- fc1: h = gelu(W1 @ yn + b1), 3-pass fp8 ----
            # m-pair groups, contraction outermost inside each pass so the
            # startup DMA stream stays ahead of the PE.  Each (m, t) PSUM
            # tile accumulates its 12 DoubleRow matmuls (3 passes x 4 k
            # pairs); a group's tiles drain while the next group computes.
            grp = {0: (w1hg0, w1lg0)}
            for g in range(KF // 2):
                ms = (2 * g, 2 * g + 1)
                if g + 1 < KF // 2:
                    grp[g + 1] = w1_group_load(g + 1, f"w1g{g + 1}")
                # stream the resident fc2 weights on a manual schedule:
                # deferred past the startup backlog (~24 us), finished
                # before fc2 needs them (~95 us).  Without the wait tags
                # the scheduler fires these dependency-free DMAs first and
                # starves the fc1 feed.
                with tc.tile_wait_until(0.024 + 0.004 * g):
                    nc.scalar.dma_start(
                        out=w2h_sb[:, 2 * g:2 * g + 2, :],
                        in_=w2h[2 * g:2 * g + 2].rearrange("a b c -> b a c"))
                with tc.tile_wait_until(0.026 + 0.004 * g):
                    nc.scalar.dma_start(
                        out=w2l_sb[:, 2 * g:2 * g + 2, :],
                        in_=w2l[2 * g:2 * g + 2].rearrange("a b c -> b a c"))

                whg, wlg = grp.pop(g)
                pss = {(mi, t): psum.tile([P, TH], f32, tag="ps",
                                          name=f"ps1_{g}_{mi}_{t}")
                       for mi in range(2) for t in range(NT)}
                # pass C (activation residual) covers 3 of 4 d-pairs:
                # with fc2's 12/16 C pass this measures rel err 0.0138
                # vs the 2e-2 gate
                for pi, (stat_sb, mov_sb) in enumerate(
                        ((whg, ynh_sb), (wlg, ynh_sb), (whg, ynl_sb))):
                    for kp in range(KD2):
                        if pi == 2 and kp == KD2 - 1:
                            continue
                        for mi in range(2):
                            stat = stat_sb[:, mi, 2 * kp:2 * kp + 2, :]
                            for t in range(NT):
                                # consecutive matmuls share the stationary
                                nc.tensor.matmul(
                                    pss[(mi, t)], stat,
                                    mov_sb[:, 2 * kp:2 * kp + 2,
                                           t * TH:(t + 1) * TH],
                                    start=(pi == 0 and kp == 0),
                                    stop=(pi == 2 and kp == KD2 - 2),
                                    perf_mode=DR,
                                )
                for mi in range(2):
                    m = ms[mi]
                    for t in range(NT):
                        col = slice(t * TH, (t + 1) * TH)
                        tmp = tmpp.tile([P, TH], bf16, tag="tmp",
                                        name=f"tmp_{g}_{mi}_{t}")
                        nc.scalar.activation(
                            tmp, pss[(mi, t)],
                            mybir.ActivationFunctionType.Gelu_apprx_tanh,
                            bias=b1_sb[:, m:m + 1], scale=1.0,
                        )
                        nc.vector.tensor_copy(hh_sb[:, m, col], tmp)
                        nc.vector.tensor_sub(hl_sb[:, m, col], tmp,
                                             hh_sb[:, m, col])

            # ---- fc2 + bias: o^T[d, t], 3-pass fp8 ----
            # d-chunk-sequential so completions stagger; the last d-chunk
            # shrinks to 128-token sub-tiles so the exposed tail after the
            # final matmul is one bias-add + one 64 KB output DMA.
            def _evict(ps, m, off, ln, alt):
                o_t = outp.tile([P, ln], f32, tag="o_t", name=f"o_{m}_{off}")
                if alt:
                    nc.scalar.activation(o_t, ps,
                                         mybir.ActivationFunctionType.Identity,
                                         bias=b2_sb[:, m:m + 1], scale=1.0)
                else:
                    nc.vector.tensor_scalar_add(o_t, ps, b2_sb[:, m:m + 1])
                eng = nc.sync if alt else nc.scalar
                eng.dma_start(out=ot[m * P:(m + 1) * P, off:off + ln], in_=o_t)

            PASSES2 = ((w2h_sb, hh_sb), (w2l_sb, hh_sb), (w2h_sb, hl_sb))
            # the activation-residual pass (C) runs on 12 of 16 k-pairs:
            # measured rel err 0.0118 vs the 0.02 gate (full C: 0.0025),
            # saving ~7 us of matmul
            C_KP2 = 12

            def fc2_tile(ps, m, off, ln):
                for pi, (stat_sb, mov_sb) in enumerate(PASSES2):
                    for kp in range(KF2):
                        if pi == 2 and kp >= C_KP2:
                            continue
                        stat = stat_sb[:, 2 * kp:2 * kp + 2,
                                       m * P:(m + 1) * P]
                        nc.tensor.matmul(
                            ps, stat,
                            mov_sb[:, 2 * kp:2 * kp + 2, off:off + ln],
                            start=(pi == 0 and kp == 0),
                            stop=(pi == 2 and kp == C_KP2 - 1),
                            perf_mode=DR,
                        )

            for m in range(KD - 1):
                ps2 = [psum.tile([P, TH], f32, tag="ps",
                                 name=f"ps2_{m}_{t}") for t in range(NT)]
                for pi, (stat_sb, mov_sb) in enumerate(PASSES2):
                    for kp in range(KF2):
                        if pi == 2 and kp >= C_KP2:
                            continue
                        stat = stat_sb[:, 2 * kp:2 * kp + 2,
                                       m * P:(m + 1) * P]
                        for t in range(NT):
                            # consecutive matmuls share the stationary
                            nc.tensor.matmul(
                                ps2[t], stat,
                                mov_sb[:, 2 * kp:2 * kp + 2,
                                       t * TH:(t + 1) * TH],
                                start=(pi == 0 and kp == 0),
                                stop=(pi == 2 and kp == C_KP2 - 1),
                                perf_mode=DR,
                            )
                for t in range(NT):
                    _evict(ps2[t], m, t * TH, TH, (m + t) % 2 == 0)

            m = KD - 1
            for i, (off, ln) in enumerate(((0, TH), (TH, 256))):
                ps2 = psum.tile([P, ln], f32, tag="ps", name=f"ps2_{m}_s{i}")
                fc2_tile(ps2, m, off, ln)
                _evict(ps2, m, off, ln, i % 2 == 0)
            # final two 128-token sub-tiles: bias-add into one shared SBUF
            # tile, ONE output DMA after both — descriptor pipeline and
            # end-of-kernel barrier dominate the tail
            o_fin = outp.tile([P, 256], f32, tag="o_fin")
            for i, off in enumerate((TH + 256, TH + 384)):
                ps2 = psum.tile([P, 128], f32, tag="ps", name=f"ps2_{m}_f{i}")
                fc2_tile(ps2, m, off, 128)
                if i == 0:
                    nc.scalar.activation(o_fin[:, 0:128], ps2,
                                         mybir.ActivationFunctionType.Identity,
                                         bias=b2_sb[:, m:m + 1], scale=1.0)
                    # ship the first half while the last sub-tile computes
                    nc.sync.dma_start(
                        out=ot[m * P:(m + 1) * P, off:off + 128],
                        in_=o_fin[:, 0:128])
                else:
                    nc.vector.tensor_scalar_add(o_fin[:, 128:256], ps2,
                                                b2_sb[:, m:m + 1])
                    nc.sync.dma_start(
                        out=ot[m * P:(m + 1) * P, off:off + 128],
                        in_=o_fin[:, 128:256])

    nc.compile()
    return nc


def get_nc():
    if "nc" not in _NC_CACHE:
        _NC_CACHE["nc"] = _build_nc()
    return _NC_CACHE["nc"]


def _gate_topk(xf32, gate_w):
    """Replicates the reference gating bit-exactly (same jnp ops, same backend)."""
    import jax
    import jax.numpy as jnp

    xf = jnp.asarray(xf32)
    gw = jnp.asarray(np.asarray(gate_w, dtype=np.float32))
    scores = xf @ gw.T
    scores = (jnp.tanh(scores) + 1.0) * 0.5
    vals, idx = jax.lax.top_k(scores.T, K_TOK)   # (n, k)
    return np.asarray(vals), np.asarray(idx)


def _hi_lo(a):
    hi = np.ascontiguousarray(a).astype(E4)
    lo = np.ascontiguousarray((a - hi.astype(np.float32)).astype(E5))
    return hi, lo


def kernel(x, gate_w, ln_gamma, ln_beta, fc1s, b1s, fc2s, b2s):
    x = np.asarray(x, dtype=np.float32)
    gate_w = np.asarray(gate_w, dtype=np.float32)
    ln_gamma = np.asarray(ln_gamma, dtype=np.float32)
    ln_beta = np.asarray(ln_beta, dtype=np.float32)
    fc1s = np.asarray(fc1s, dtype=np.float32)
    b1s = np.asarray(b1s, dtype=np.float32)
    fc2s = np.asarray(fc2s, dtype=np.float32)
    b2s = np.asarray(b2s, dtype=np.float32)

    og_shape = x.shape
    xf = x.reshape(-1, D)
    vals, idx = _gate_topk(xf, gate_w)

    np_inputs = {"ln_gamma": ln_gamma, "ln_beta": ln_beta,
                 "fc1s": fc1s, "b1s": b1s, "fc2s": fc2s, "b2s": b2s}
    in_maps = build_in_maps(np_inputs, xf, vals, idx)

    nc = get_nc()
    res = run_bass_kernel_spmd(nc, in_maps, core_ids=list(range(N_EXP)))

    out = xf.copy()
    for e in range(N_EXP):
        o_e = np.asarray(res.results[e]["ot"]).T           # (k, d) f32
        out[idx[e]] += o_e * vals[e][:, None]
    return out.reshape(og_shape)


def build_in_maps(np_inputs, xf, vals, idx):
    gam = np_inputs["ln_gamma"]
    bet = np_inputs["ln_beta"]
    maps = []
    for e in range(N_EXP):
        y_e = xf[idx[e]]                                   # (k, d) f32
        mu = y_e.mean(axis=1, keepdims=True)
        var = y_e.var(axis=1, keepdims=True)
        yn = (y_e - mu) / np.sqrt(var + LN_EPS) * gam + bet
        ynt = np.ascontiguousarray(yn.T).reshape(KD, P, K_TOK)
        ynh_, ynl_ = _hi_lo(ynt)
        w1r = np.ascontiguousarray(
            np_inputs["fc1s"][e].reshape(KF, P, KD, P).transpose(0, 3, 2, 1)
        ).reshape(KF, P, KD * P)
        w1h_, w1l_ = _hi_lo(w1r)
        w2r = np.ascontiguousarray(
            np_inputs["fc2s"][e].reshape(KD, P, KF, P).transpose(2, 3, 0, 1)
        ).reshape(KF, P, KD * P)
        w2h_, w2l_ = _hi_lo(w2r)
        maps.append({
            "ynh": ynh_, "ynl": ynl_,
            "w1h": w1h_, "w1l": w1l_,
            "w2h": w2h_, "w2l": w2l_,
            "b1r": np.ascontiguousarray(np_inputs["b1s"][e].reshape(KF, P).T),
            "b2r": np.ascontiguousarray(np_inputs["b2s"][e].reshape(KD, P).T),
        })
    return maps
